# revision 21
# baseline (speedup 1.0000x reference)
"""Trainium2 Bass kernel for nn_AttentionBlock (GroupNorm + attention block),
data-parallel over batch across 8 NeuronCores (one batch element per core).

v3 design notes:
  - S_T head-PAIR packing: heads (2t, 2t+1) occupy PE row groups 0-63 /
    64-127, so their K=64 QK^T matmuls run concurrently (~2x).
  - qk and S interleaved per pair so softmax exp (the ACT/DVE-bound part)
    starts ~18us in and overlaps the whole qkv phase. pT is 4-deep.
  - Softmax exp split across ScalarE (table exp) and VectorE (Schraudolph
    bitcast exp: int16(A*x+B) reinterpreted as bf16, one tensor_scalar).
  - AV lhsT per head = contiguous [v_h | ones] 128 cols; output rows 0-63 =
    ha, rows 64-127 = Z replicated (free-dim bound: the ones cost nothing).
  - Normalization: reciprocal_approx_fast straight off the PSUM Z rows into
    a [P, N] zinv tile (head a -> rows 64-127, head b -> rows 0-63), then
    one fused tensor_mul eviction per head. No gather, no broadcast matmul.
  - k bias dropped (softmax-row invariant); v bias folded into b_proj on
    the host; q bias applied in the PSUM eviction.
  - Weights pre-arranged on host to the SBUF layout so the f32->bf16
    casting DMAs (gpsimd queue) are contiguous; wqk split per pair so
    pair-0 weights land first.
  - PE warm-keeper dummy matmuls with data deps placed to bridge the DMA +
    GroupNorm stats window (keeps HAM at K=8/8).
"""

import os

import numpy as np

import concourse.bass as bass
import concourse.bacc as bacc
import concourse.mybir as mybir
import concourse.tile as tile
from concourse.bass_utils import run_bass_kernel_spmd

F32 = mybir.dt.float32
BF16 = mybir.dt.bfloat16
I16 = mybir.dt.int16
AF = mybir.ActivationFunctionType
ALU = mybir.AluOpType

B = 8
C = 512
N = 1024          # H*W = 32*32
H = 8             # num heads
HD = 64           # head dim
G = 32            # groups
GS = C // G       # channels per group = 16
CCH = 4           # channel chunks of 128
NT = 2            # n tiles of 512
MT = 8            # m tiles of 128
PAIRS = 4
EPS = 1e-5
P = 128
NCORES = 8
SCALE = float(HD) ** -0.5

# Schraudolph bf16 exp: bitcast_bf16(int16(A*(scale*s) + B)) ~ exp(scale*s)
A_EXP = (128.0 / float(np.log(2.0))) * SCALE
B_EXP = 16250.875

# per-pair ACT share of the 16 exp granules (rest -> VectorE Schraudolph)
EXP_ACT = [int(v) for v in os.environ.get("EXP_ACT", "11,12,12,14").split(",")]
KW1 = int(os.environ.get("KW1", "24"))
KW2 = int(os.environ.get("KW2", "12"))

_CACHE = {}


def build_nc():
    nc = bacc.Bacc(
        "TRN2", target_bir_lowering=False, debug=False, num_devices=NCORES
    )

    x_d = nc.declare_dram_parameter("x", [C, N], F32, isOutput=False)
    # host pre-arranged: [P, CCH, ...] layouts, contiguous DMA targets
    wqk_d = nc.declare_dram_parameter("w_qkr", [P, PAIRS, 2, CCH, P], F32, isOutput=False)
    bq_d = nc.declare_dram_parameter("b_q", [P, CCH], F32, isOutput=False)
    wv_d = nc.declare_dram_parameter("w_vr", [P, CCH, C], F32, isOutput=False)
    wp_d = nc.declare_dram_parameter("w_pr", [P, CCH, C], F32, isOutput=False)
    bp_d = nc.declare_dram_parameter("b_p2", [P, CCH], F32, isOutput=False)
    gm_d = nc.declare_dram_parameter("gamma", [P, CCH], F32, isOutput=False)
    bt_d = nc.declare_dram_parameter("beta", [P, CCH], F32, isOutput=False)
    gsel_d = nc.declare_dram_parameter("gsel", [P, 8], F32, isOutput=False)
    gselT_d = nc.declare_dram_parameter("gselT", [8, P], F32, isOutput=False)
    dums_d = nc.declare_dram_parameter("dums", [P, 512], BF16, isOutput=False)
    out_d = nc.declare_dram_parameter("out", [C, N], F32, isOutput=True)
    dbg = bool(os.environ.get("KDBG"))
    if dbg:
        dxn_d = nc.declare_dram_parameter("d_xn", [P, CCH, N], F32, isOutput=True)
        dqk_d = nc.declare_dram_parameter("d_qk", [P, 8, N], F32, isOutput=True)
        dvt_d = nc.declare_dram_parameter("d_vt", [P, MT, 8, P], F32, isOutput=True)
        dpt_d = nc.declare_dram_parameter(
            "d_pt", [P, MT, NT, 2, 512], F32, isOutput=True
        )
        dha_d = nc.declare_dram_parameter("d_ha", [P, CCH, N], F32, isOutput=True)

    with tile.TileContext(nc) as tc:
        with (
            tc.tile_pool(name="singles", bufs=1) as singles,
            tc.tile_pool(name="pa", bufs=3, space="PSUM") as pa,
            tc.tile_pool(name="pav", bufs=2, space="PSUM") as pav,
        ):
            # ---------------- static tiles ----------------
            x_sb = singles.tile([P, CCH, N], F32)
            sq_scr = singles.tile([P, N], F32)
            wqk_sb = singles.tile([P, PAIRS, 2, CCH, P], BF16)
            wv_sb = singles.tile([P, CCH, C], BF16)
            wp_sb = singles.tile([P, CCH, C], BF16)
            bq_sb = singles.tile([P, CCH], F32)
            bp_sb = singles.tile([P, CCH], F32)
            gm_sb = singles.tile([P, CCH], F32)
            bt_sb = singles.tile([P, CCH], F32)
            gsel_sb = singles.tile([P, 8], BF16)
            gselT_sb = singles.tile([8, P], BF16)
            dums = singles.tile([P, 512], BF16)

            xn_sb = singles.tile([P, CCH, N], BF16)
            qk_sb = singles.tile([P, 8, N], BF16)   # slot t<4: q pair t; 4+t: k
            vT_sb = singles.tile([P, MT, 8, P], BF16)  # per head: [v_h | ones]
            pT_t = [
                singles.tile([P, MT, NT, 2, 512], BF16, name=f"pT{i}")
                for i in range(2)
            ]
            zinv_t = [
                singles.tile([64, N], F32, name=f"zinv{i}") for i in range(2)
            ]
            zs_t = [
                singles.tile([64, N], F32, name=f"zs{i}") for i in range(2)
            ]
            ha_sb = singles.tile([P, CCH, N], BF16)
            out_sb = singles.tile([P, CCH, N], F32)

            s12_sb = singles.tile([P, CCH, 2], F32)   # per cc: (sum, sumsq)
            s12_bf = singles.tile([P, CCH, 2], BF16)
            mu_rs = singles.tile([8, 8], F32)       # cols 0-3 mu, 4-7 rs
            mu_rs_bf = singles.tile([8, 8], BF16)
            tmp8 = singles.tile([8, CCH], F32)
            var_sb = singles.tile([8, CCH], F32)
            lnv_sb = singles.tile([8, CCH], F32)
            s0_sb = singles.tile([P, CCH], F32)
            tmp128 = singles.tile([P, CCH], F32)
            sbias_sb = singles.tile([P, CCH], F32)
            eps_sb = singles.tile([8, 1], F32)

            # ---------------- input DMAs ----------------
            nc.sync.dma_start(dums[:], dums_d.ap())
            x_v = x_d.ap().rearrange("(cc p) n -> p cc n", p=P)
            for cc in range(2):
                nc.sync.dma_start(x_sb[:, cc, :], x_v[:, cc, :])
            for cc in range(2, CCH):
                nc.gpsimd.dma_start(x_sb[:, cc, :], x_v[:, cc, :])
            nc.sync.dma_start(bq_sb[:], bq_d.ap())
            nc.sync.dma_start(bp_sb[:], bp_d.ap())
            nc.sync.dma_start(gm_sb[:], gm_d.ap())
            nc.sync.dma_start(bt_sb[:], bt_d.ap())
            # casting DMAs (f32 dram -> bf16 sbuf) on the gpsimd queue;
            # wqk split per pair so pair 0's q+k weights land first
            for t in range(PAIRS):
                nc.gpsimd.dma_start(wqk_sb[:, t], wqk_d.ap()[:, t])
            nc.gpsimd.dma_start(wv_sb[:], wv_d.ap())
            nc.gpsimd.dma_start(gsel_sb[:], gsel_d.ap())
            nc.gpsimd.dma_start(gselT_sb[:], gselT_d.ap())
            nc.gpsimd.dma_start(wp_sb[:], wp_d.ap())

            nc.vector.memset(eps_sb[:], EPS)
            nc.vector.memset(vT_sb[:, :, :, 64:128], 1.0)

            # PE warm-keeper: dummy matmuls (dums arrives by DMA at t~1us)
            if KW1 > 0:
                ps_w = pa.tile([P, N], F32, tag="ps")
                for _ in range(KW1):
                    nc.tensor.matmul(
                        ps_w[0:P, 0:512], dums[:, 0:P], dums[:], start=True,
                        stop=True,
                    )

            # ---------------- GroupNorm stats (per-chunk, no barrier) ----
            ps_st = pa.tile([P, N], F32, tag="ps")
            for cc in range(CCH):
                nc.vector.reduce_sum(
                    s12_sb[:, cc, 0:1], x_sb[:, cc, :],
                    axis=mybir.AxisListType.X,
                )
                nc.scalar.activation(
                    sq_scr[:], x_sb[:, cc, :], AF.Square,
                    accum_out=s12_sb[:, cc, 1:2],
                )
                nc.vector.tensor_copy(s12_bf[:, cc, :], s12_sb[:, cc, :])
                nc.tensor.matmul(
                    ps_st[0:8, 2 * cc : 2 * cc + 2], gsel_sb[:],
                    s12_bf[:, cc, :], start=True, stop=True,
                )
            # warm-keeper #2: depends on s12_bf so it lands in the stats gap
            if KW2 > 0:
                ps_w2 = pa.tile([P, N], F32, tag="ps")
                for _ in range(KW2):
                    nc.tensor.matmul(
                        ps_w2[0:2, 0:512], s12_bf[0:2, 0, :], dums[0:2, :],
                        start=True, stop=True,
                    )
            inv_cnt = 1.0 / (GS * N)
            # mu = s1/cnt ; var = s2/cnt - mu^2 ; rs = exp(-0.5*ln(var+eps))
            s1v = ps_st[0:8, 0:8].rearrange("g (cc two) -> g cc two", two=2)
            nc.vector.tensor_scalar_mul(mu_rs[:, 0:4], s1v[:, :, 0], inv_cnt)
            nc.vector.tensor_mul(tmp8[:], mu_rs[:, 0:4], mu_rs[:, 0:4])
            nc.vector.scalar_tensor_tensor(
                out=var_sb[:],
                in0=s1v[:, :, 1],
                scalar=inv_cnt,
                in1=tmp8[:],
                op0=ALU.mult,
                op1=ALU.subtract,
            )
            nc.scalar.activation(lnv_sb[:], var_sb[:], AF.Ln, bias=eps_sb[:])
            nc.scalar.activation(mu_rs[:, 4:8], lnv_sb[:], AF.Exp, scale=-0.5)
            nc.vector.tensor_copy(mu_rs_bf[:], mu_rs[:])
            ps_bc = pa.tile([P, N], F32, tag="ps")
            nc.tensor.matmul(
                ps_bc[0:P, 0:8], gselT_sb[:], mu_rs_bf[:], start=True, stop=True
            )
            nc.vector.tensor_mul(s0_sb[:], ps_bc[0:P, 4:8], gm_sb[:])
            nc.vector.tensor_mul(tmp128[:], ps_bc[0:P, 0:4], s0_sb[:])
            nc.vector.tensor_sub(sbias_sb[:], bt_sb[:], tmp128[:])
            for cc in range(CCH):
                nc.vector.tensor_scalar(
                    out=xn_sb[:, cc, :],
                    in0=x_sb[:, cc, :],
                    scalar1=s0_sb[:, cc : cc + 1],
                    scalar2=sbias_sb[:, cc : cc + 1],
                    op0=ALU.mult,
                    op1=ALU.add,
                )

            # ---------------- qkv + attention, granule-interleaved ----------
            # Generators emit one PE work quantum per next(); pump() meshes
            # them so S-granule exp evictions (ACT/DVE-bound) never back up
            # the PSUM pool while qk/v/AV matmuls keep the PE dense.
            def gen_qk(t):
                for j in (0, 1):
                    slot = t if j == 0 else 4 + t
                    ps_qk = pa.tile([P, N], F32, tag="ps", name=f"qk{slot}")
                    for nt in range(NT):
                        for cc in range(CCH):
                            nc.tensor.matmul(
                                ps_qk[:, nt * 512 : (nt + 1) * 512],
                                wqk_sb[:, t, j, cc, :],
                                xn_sb[:, cc, nt * 512 : (nt + 1) * 512],
                                start=(cc == 0),
                                stop=(cc == CCH - 1),
                            )
                            if nt * CCH + cc < NT * CCH - 1:
                                yield
                    if j == 0:  # q: bias folded into the eviction
                        nc.scalar.activation(
                            qk_sb[:, slot, :], ps_qk[:], AF.Identity,
                            bias=bq_sb[:, t : t + 1],
                        )
                    else:       # k: bias dropped (softmax-row invariant)
                        nc.scalar.activation(
                            qk_sb[:, slot, :], ps_qk[:], AF.Identity
                        )
                    yield

            def gen_s(t):
                act_of16 = EXP_ACT[t % len(EXP_ACT)]
                pT = pT_t[t % 2]
                for mt in range(MT):
                    for nt in range(NT):
                        ps = pa.tile(
                            [P, N], F32, tag="ps", name=f"s{t}_{mt}_{nt}"
                        )
                        for hh in range(2):
                            po = 64 * hh
                            nc.tensor.matmul(
                                ps[:, hh * 512 : (hh + 1) * 512],
                                qk_sb[po : po + 64, 4 + t, mt * P : (mt + 1) * P],
                                qk_sb[po : po + 64, t, nt * 512 : (nt + 1) * 512],
                                start=True,
                                stop=True,
                            )
                        g = mt * NT + nt
                        dst = pT[:, mt, nt, :, :]
                        if (g * act_of16) % 16 < act_of16:
                            nc.scalar.activation(dst, ps[:], AF.Exp, scale=SCALE)
                        else:
                            nc.vector.tensor_scalar(
                                out=dst.bitcast(I16),
                                in0=ps[:],
                                scalar1=A_EXP,
                                scalar2=B_EXP,
                                op0=ALU.mult,
                                op1=ALU.add,
                            )
                        yield

            def gen_v():
                for mt in range(MT):
                    ps_v = pa.tile([P, N], F32, tag="ps", name=f"v{mt}")
                    for cc in range(CCH):
                        nc.tensor.matmul(
                            ps_v[:, 0:C],
                            xn_sb[:, cc, mt * P : (mt + 1) * P],
                            wv_sb[:, cc, :],
                            start=(cc == 0),
                            stop=(cc == CCH - 1),
                        )
                        if cc < CCH - 1:
                            yield
                    nc.scalar.activation(
                        vT_sb[:, mt, :, 0:64], ps_v[:, 0:C], AF.Identity
                    )
                    yield

            def gen_av(t):
                pT = pT_t[t % 2]
                for hh in range(2):
                    po = 64 * hh
                    zinv = zinv_t[hh]
                    zs = zs_t[hh]
                    for nt in range(NT):
                        ns = slice(nt * 512, (nt + 1) * 512)
                        ps_av = pav.tile(
                            [P, 512], F32, tag="av", name=f"av{t}_{hh}_{nt}"
                        )
                        for mt in range(MT):
                            nc.tensor.matmul(
                                ps_av[:, :],
                                vT_sb[:, mt, 2 * t + hh, :],
                                pT[:, mt, nt, hh, :],
                                start=(mt == 0),
                                stop=(mt == MT - 1),
                            )
                            if mt < MT - 1:
                                yield
                        nc.vector.tensor_copy(zs[:, ns], ps_av[64:128, :])
                        nc.vector.reciprocal_approx_fast(
                            out=zinv[:, ns], in_=zs[:, ns]
                        )
                        nc.vector.tensor_mul(
                            ha_sb[po : po + 64, t, ns],
                            ps_av[0:64, :],
                            zinv[:, ns],
                        )
                        yield

            def pump(*streams):
                # streams: (generator, quanta_per_round); round-robin mesh
                streams = [[g, n] for g, n in streams]
                while streams:
                    for s in list(streams):
                        g, n = s
                        for _ in range(n):
                            try:
                                next(g)
                            except StopIteration:
                                streams.remove(s)
                                break

            pump((gen_qk(0), 4))
            pump((gen_qk(1), 1), (gen_s(0), 1))
            pump((gen_v(), 2), (gen_s(1), 1))
            pump((gen_qk(2), 1), (gen_av(0), 2))
            pump((gen_qk(3), 1), (gen_s(2), 1), (gen_av(1), 2))
            pump((gen_s(3), 1), (gen_av(2), 2))
            pump((gen_av(3), 4))

            if dbg:
                nc.gpsimd.dma_start(dxn_d.ap(), xn_sb[:])
                nc.gpsimd.dma_start(dqk_d.ap(), qk_sb[:])
                nc.gpsimd.dma_start(dvt_d.ap(), vT_sb[:])
                nc.gpsimd.dma_start(dpt_d.ap(), pT_t[0][:])
                nc.gpsimd.dma_start(dha_d.ap(), ha_sb[:])

            # ---------------- proj + bias + residual ----------------
            out_v = out_d.ap().rearrange("(ot p) n -> p ot n", p=P)
            for ot in range(CCH):
                ps_p = pa.tile([P, N], F32, tag="ps", name=f"p{ot}")
                for nt in range(NT):
                    for cc in range(CCH):
                        nc.tensor.matmul(
                            ps_p[:, nt * 512 : (nt + 1) * 512],
                            wp_sb[:, cc, ot * P : (ot + 1) * P],
                            ha_sb[:, cc, nt * 512 : (nt + 1) * 512],
                            start=(cc == 0),
                            stop=(cc == CCH - 1),
                        )
                nc.vector.scalar_tensor_tensor(
                    out=out_sb[:, ot, :],
                    in0=ps_p[:],
                    scalar=bp_sb[:, ot : ot + 1],
                    in1=x_sb[:, ot, :],
                    op0=ALU.add,
                    op1=ALU.add,
                )
                nc.sync.dma_start(out_v[:, ot, :], out_sb[:, ot, :])

    nc.compile()
    return nc


def make_in_maps(x, gn_gamma, gn_beta, w_qkv, b_qkv, w_proj, b_proj):
    f32 = np.float32
    w_qkv = np.asarray(w_qkv, dtype=f32)
    b_qkv = np.asarray(b_qkv, dtype=f32)
    w_proj = np.asarray(w_proj, dtype=f32)
    b_proj = np.asarray(b_proj, dtype=f32)
    b_v = b_qkv[2 * C :]
    bp2 = b_proj + w_proj @ b_v

    def rearr(wT):  # [C(in), O] -> [P, CCH, O] with in-channel = cc*128 + p
        return np.ascontiguousarray(wT.reshape(CCH, P, -1).transpose(1, 0, 2))

    shared = {
        "w_qkr": np.ascontiguousarray(
            w_qkv[: 2 * C].T.reshape(CCH, P, 2, PAIRS, P)
            .transpose(1, 3, 2, 0, 4)
        ),
        "b_q": np.ascontiguousarray(b_qkv[:C].reshape(CCH, P).T),
        "w_vr": rearr(w_qkv[2 * C :].T),
        "w_pr": rearr(w_proj.T),
        "b_p2": np.ascontiguousarray(bp2.reshape(CCH, P).T),
        "gamma": np.ascontiguousarray(
            np.asarray(gn_gamma, dtype=f32).reshape(CCH, P).T
        ),
        "beta": np.ascontiguousarray(
            np.asarray(gn_beta, dtype=f32).reshape(CCH, P).T
        ),
        "dums": np.full((P, 512), 0.02, __import__("ml_dtypes").bfloat16),
    }
    gsel = np.zeros((P, 8), f32)
    for p in range(P):
        gsel[p, p // GS] = 1.0
    shared["gsel"] = gsel
    shared["gselT"] = np.ascontiguousarray(gsel.T)
    in_maps = []
    for b in range(B):
        m = dict(shared)
        m["x"] = np.ascontiguousarray(np.asarray(x[b], dtype=f32).reshape(C, N))
        in_maps.append(m)
    return in_maps


def kernel(x, gn_gamma, gn_beta, w_qkv, b_qkv, w_proj, b_proj):
    if "nc" not in _CACHE:
        _CACHE["nc"] = build_nc()
    nc = _CACHE["nc"]
    in_maps = make_in_maps(x, gn_gamma, gn_beta, w_qkv, b_qkv, w_proj, b_proj)
    trace = bool(os.environ.get("KERNEL_TRACE"))
    res = run_bass_kernel_spmd(
        nc, in_maps, core_ids=list(range(NCORES)), trace=trace
    )
    _CACHE["last_result"] = res
    out = np.stack([np.asarray(res.results[i]["out"]) for i in range(NCORES)])
    return out.reshape(B, C, 32, 32).astype(np.float32)
